# revision 4
# baseline (speedup 1.0000x reference)
"""FFM layer kernel for Trainium2 (8 NeuronCores, data-parallel over batch).

Math (reference):
  idx[b,j]  = 13 + j*10000 + sparse_x[b,j]                 (26 sparse fields)
  linear    = dense_x @ w[:13] + sum_j w[idx] + w0         (B,1)
  field_f   = einsum('bd,dfk', dense_x, v[:13]) + sum_j v[idx]   (B,39,8)
  s         = sum_f field_f                                 (B,8)
  cross     = 0.5*(sum_k s^2 - sum_{f,k} field_f^2)
  out       = sigmoid(linear + cross)

Device strategy (per core, 512 samples):
  - Table rows augmented+padded on host: row r = [v[r].flat(312), w[r], 0...]
    (320 f32 = 1280 B, 64B-aligned, %256==0 for dma_gather).
  - 26 dma_gather ops (one per sparse field, 512 int16 indices each) fetch rows
    from the field's 10000-row table slice into [128, 4, 320] tiles; sample s
    lands at [s%128, s//128, :].  DVE accumulates each landed tile into a
    [128, 4*320] running sum, overlapped with the next gather's transfer.
  - Dense contribution (and the linear term + w0) via one PE matmul per chunk:
    lhsT = dense_x^T chunk padded to [128,128] with an all-ones row 13,
    rhs[d] = [v[d].flat, w[d], 0...] and rhs[13,312] = w0.
  - DVE: strided reductions for sum_f, fused square-reduce for the cross term.
  - ACT: sigmoid.
"""

import numpy as np

N_DENSE = 13
N_SPARSE = 26
VOCAB = 10000
N_FIELD = 39
N_FEAT = N_DENSE + N_SPARSE * VOCAB  # 260013
K = 8
ROW = N_FIELD * K  # 312
ROWP = 320  # padded row (f32 elems) -> 1280 B
BATCH = 4096
N_CORES = 8
BC = BATCH // N_CORES  # 512 per core
P = 128
NCHUNK = BC // P  # 4
IDXC = BC // 16  # 32 int16 index columns per field

_CACHE: dict = {}


def _build_program():
    import concourse.bacc as bacc
    import concourse.tile as tile
    import concourse.mybir as mybir

    f32 = mybir.dt.float32
    i16 = mybir.dt.int16

    import os as _os

    _nq = int(_os.environ.get("K_NQUEUES", "1"))
    _scratch = int(_os.environ.get("K_SCRATCH", "16384"))
    nc = bacc.Bacc(
        "TRN2",
        target_bir_lowering=False,
        debug=False,
        num_swdge_queues=_nq,
        dynamic_dma_scratch_size=_scratch,
    )

    table = nc.dram_tensor("table", [N_FEAT, ROWP], f32, kind="ExternalInput")
    xt = nc.dram_tensor("xt", [P, BC], f32, kind="ExternalInput")
    vd = nc.dram_tensor("vd", [P, ROWP], f32, kind="ExternalInput")
    idx16 = nc.dram_tensor("idx16", [P, N_SPARSE * IDXC], i16, kind="ExternalInput")
    out = nc.dram_tensor("out", [BC, 1], f32, kind="ExternalOutput")

    with tile.TileContext(nc) as tc:
        with (
            tc.tile_pool(name="const", bufs=1) as cpool,
            tc.tile_pool(
                name="gather", bufs=int(__import__("os").environ.get("K_GBUFS", "4"))
            ) as gpool,
            tc.tile_pool(name="work", bufs=2) as wpool,
            tc.tile_pool(name="psum", bufs=2, space="PSUM") as ppool,
        ):
            xt_sb = cpool.tile([P, BC], f32)
            nc.sync.dma_start(out=xt_sb[:], in_=xt[:])
            vd_sb = cpool.tile([P, ROWP], f32)
            nc.sync.dma_start(out=vd_sb[:], in_=vd[:])
            idx_sb = cpool.tile([P, N_SPARSE * IDXC], i16)
            nc.sync.dma_start(out=idx_sb[:], in_=idx16[:])

            # running sum over fields of the gathered rows, all 4 chunks wide
            acc = cpool.tile([P, NCHUNK * ROWP], f32)

            for j in range(N_SPARSE):
                g = gpool.tile([P, NCHUNK, ROWP], f32, tag="g")
                base = N_DENSE + j * VOCAB
                import os as _os

                _sp = _os.environ.get("K_SINGLE_PACKET", "1") == "1"
                _nq = int(_os.environ.get("K_NQUEUES", "1"))
                nc.gpsimd.dma_gather(
                    out_ap=g[:],
                    in_ap=table[base:base + VOCAB, :],
                    idxs_ap=idx_sb[:, j * IDXC:(j + 1) * IDXC],
                    num_idxs=BC,
                    num_idxs_reg=BC,
                    elem_size=ROWP,
                    single_packet=_sp,
                    queue_num=j % _nq,
                )
                gf = g[:].rearrange("p c m -> p (c m)")
                if j == 0:
                    nc.vector.tensor_copy(out=acc[:], in_=gf)
                else:
                    nc.vector.tensor_tensor(
                        out=acc[:], in0=acc[:], in1=gf, op=mybir.AluOpType.add
                    )

            for c in range(NCHUNK):
                psum = ppool.tile([P, ROWP], f32, space="PSUM")
                nc.tensor.matmul(
                    out=psum[:],
                    lhsT=xt_sb[:, c * P:(c + 1) * P],
                    rhs=vd_sb[:],
                    start=True,
                    stop=True,
                )

                field = wpool.tile([P, ROWP], f32)
                nc.vector.tensor_tensor(
                    out=field[:],
                    in0=acc[:, c * ROWP:(c + 1) * ROWP],
                    in1=psum[:],
                    op=mybir.AluOpType.add,
                )

                # q = sum(field[:, :312]^2)   (square on ACT, reduce on DVE)
                sq = wpool.tile([P, ROW], f32)
                nc.scalar.square(sq[:], field[:, :ROW])
                q = wpool.tile([P, 1], f32)
                nc.vector.reduce_sum(out=q[:], in_=sq[:], axis=mybir.AxisListType.X)

                # s[k] = sum_f field[f*8+k]: view [P, K, N_FIELD], reduce X
                s = wpool.tile([P, K], f32)
                fv = field[:, :ROW].rearrange("p (f k) -> p k f", f=N_FIELD)
                nc.vector.reduce_sum(out=s[:], in_=fv, axis=mybir.AxisListType.X)

                ss = wpool.tile([P, K], f32)
                nc.vector.tensor_tensor(
                    out=ss[:], in0=s[:], in1=s[:], op=mybir.AluOpType.mult
                )
                ssum = wpool.tile([P, 1], f32)
                nc.vector.reduce_sum(
                    out=ssum[:], in_=ss[:], axis=mybir.AxisListType.X
                )
                d = wpool.tile([P, 1], f32)
                nc.vector.tensor_tensor(
                    out=d[:], in0=ssum[:], in1=q[:], op=mybir.AluOpType.subtract
                )

                # out = sigmoid(0.5*d + linear)  (linear incl. w0 = field col 312)
                oc = wpool.tile([P, 1], f32)
                nc.scalar.activation(
                    oc[:],
                    d[:],
                    mybir.ActivationFunctionType.Sigmoid,
                    bias=field[:, ROW:ROW + 1],
                    scale=0.5,
                )
                nc.sync.dma_start(out=out[c * P:(c + 1) * P, :], in_=oc[:])

    nc.compile()
    return nc


def _prep_inputs(dense_x, sparse_x, w0, w, v):
    table = np.zeros((N_FEAT, ROWP), dtype=np.float32)
    table[:, :ROW] = v.reshape(N_FEAT, ROW)
    table[:, ROW] = w[:, 0]

    vd = np.zeros((P, ROWP), dtype=np.float32)
    vd[:N_DENSE, :ROW] = v[:N_DENSE].reshape(N_DENSE, ROW)
    vd[:N_DENSE, ROW] = w[:N_DENSE, 0]
    vd[N_DENSE, ROW] = np.float32(w0[0])

    xt_full = np.zeros((P, BATCH), dtype=np.float32)
    xt_full[:N_DENSE] = dense_x.T
    xt_full[N_DENSE] = 1.0

    in_maps = []
    for r in range(N_CORES):
        b0 = r * BC
        sp = sparse_x[b0:b0 + BC].astype(np.int16)  # values < 10000 fit
        idx16 = np.zeros((P, N_SPARSE * IDXC), dtype=np.int16)
        for j in range(N_SPARSE):
            # gather position g reads idx[g%16, g//16]; g == sample index.
            # Replicated across all 8 GPSIMD-core partition groups (HW reads
            # its own 16-partition window).
            blk = sp[:, j].reshape(IDXC, 16).T
            idx16[:, j * IDXC:(j + 1) * IDXC] = np.tile(blk, (P // 16, 1))
        in_maps.append(
            {
                "table": table,
                "xt": np.ascontiguousarray(xt_full[:, b0:b0 + BC]),
                "vd": vd,
                "idx16": idx16,
            }
        )
    return in_maps


def kernel(dense_x, sparse_x, w0, w, v, _trace=False, _trace_kwargs=None):
    from concourse.bass_utils import run_bass_kernel_spmd

    if "nc" not in _CACHE:
        _CACHE["nc"] = _build_program()
    nc = _CACHE["nc"]

    in_maps = _prep_inputs(dense_x, sparse_x, w0, w, v)
    kw = {}
    if _trace:
        kw["trace"] = True
        if _trace_kwargs:
            kw.update(_trace_kwargs)
    res = run_bass_kernel_spmd(nc, in_maps, core_ids=list(range(N_CORES)), **kw)
    outs = [res.results[r]["out"] for r in range(N_CORES)]
    full = np.concatenate(outs, axis=0).astype(np.float32)
    if _trace:
        _CACHE["last_exec_time_ns"] = res.exec_time_ns
        _CACHE["last_results"] = res
    return full



# revision 5
# speedup vs baseline: 1.0970x; 1.0970x over previous
"""FFM layer kernel for Trainium2 (8 NeuronCores, data-parallel over batch).

Math (reference):
  idx[b,j]  = 13 + j*10000 + sparse_x[b,j]                 (26 sparse fields)
  linear    = dense_x @ w[:13] + sum_j w[idx] + w0         (B,1)
  field_f   = einsum('bd,dfk', dense_x, v[:13]) + sum_j v[idx]   (B,39,8)
  s         = sum_f field_f                                 (B,8)
  cross     = 0.5*(sum_k s^2 - sum_{f,k} field_f^2)
  out       = sigmoid(linear + cross)

Device strategy (per core, 512 samples):
  - Table rows fp16, augmented+padded on host: row r = [v[r].flat(312), w[r], 0...]
    (384 f16 = 768 B, %256==0 for dma_gather).
  - 26 dma_gather ops (one per sparse field, 512 int16 indices each) spread
    round-robin over 4 SWDGE queues (one Q7 core-pair each) so descriptor
    generation overlaps across core pairs.
  - Accumulation of the 26 gathered tiles runs on the PE (identity-weight
    matmuls accumulating in PSUM, fp32) -- NOT on DVE.  DVE 2-port perf-mode
    ops lock GPSIMD out of the shared SBUF port and starve SWDGE descriptor
    generation; PE has its own SBUF read ports and a dedicated PSUM write
    port, so gathers and accumulation are fully independent.
  - Dense contribution (and the linear term + w0) accumulates into the same
    PSUM tiles: lhsT = dense_x^T chunk (fp16) with an all-ones row 13,
    rhs[d] = [v[d].flat, w[d], 0...] and rhs[13,312] = w0.
  - Tail per chunk on DVE/ACT: fused square-reduce (tensor_tensor_reduce),
    strided field-sum reduce, sigmoid.
  - A dummy 128-idx gather at t=0 (zeroed idx via gpsimd.memset) fronts the
    ~6us GPSIMD IRAM ucode load so it overlaps the input DMAs.
"""

import os

import numpy as np

N_DENSE = 13
N_SPARSE = 26
VOCAB = 10000
N_FIELD = 39
N_FEAT = N_DENSE + N_SPARSE * VOCAB  # 260013
K = 8
ROW = N_FIELD * K  # 312
ROWP = 384  # padded row (f16 elems) -> 768 B, %256==0
BATCH = 4096
N_CORES = 8
BC = BATCH // N_CORES  # 512 per core
P = 128
NCHUNK = BC // P  # 4
IDXC = BC // 16  # 32 int16 index columns per field

_CACHE: dict = {}


def _build_program():
    import concourse.bacc as bacc
    import concourse.tile as tile
    import concourse.mybir as mybir

    f32 = mybir.dt.float32
    f16 = mybir.dt.float16
    i16 = mybir.dt.int16

    nq = int(os.environ.get("K_NQUEUES", "4"))
    gbufs = int(os.environ.get("K_GBUFS", "8"))
    scratch = int(os.environ.get("K_SCRATCH", "16384"))

    nc = bacc.Bacc(
        "TRN2",
        target_bir_lowering=False,
        debug=False,
        num_swdge_queues=nq,
        dynamic_dma_scratch_size=scratch,
    )

    table = nc.dram_tensor("table", [N_FEAT, ROWP], f16, kind="ExternalInput")
    xt = nc.dram_tensor("xt", [P, BC], f16, kind="ExternalInput")
    vd = nc.dram_tensor("vd", [P, ROWP], f16, kind="ExternalInput")
    idx16 = nc.dram_tensor("idx16", [P, N_SPARSE * IDXC], i16, kind="ExternalInput")
    ident = nc.dram_tensor("ident", [P, P], f16, kind="ExternalInput")
    out = nc.dram_tensor("out", [BC, 1], f32, kind="ExternalOutput")

    with tile.TileContext(nc) as tc:
        with (
            tc.tile_pool(name="const", bufs=1) as cpool,
            tc.tile_pool(name="gather", bufs=gbufs) as gpool,
            tc.tile_pool(name="work", bufs=2) as wpool,
            tc.tile_pool(name="psum", bufs=1, space="PSUM") as ppool,
        ):
            # Dummy gather first: zero idx via memset (Pool engine, no DMA
            # dependency) so the GPSIMD ext-isa IRAM load overlaps input DMAs.
            dummy_idx = cpool.tile([P, 8], i16)
            nc.gpsimd.memset(dummy_idx[:], 0)
            dummy_g = cpool.tile([P, 1, ROWP], f16)
            nc.gpsimd.dma_gather(
                out_ap=dummy_g[:],
                in_ap=table[0:16, :],
                idxs_ap=dummy_idx[:],
                num_idxs=P,
                num_idxs_reg=P,
                elem_size=ROWP,
                single_packet=True,
                queue_num=0,
            )

            idx_sb = cpool.tile([P, N_SPARSE * IDXC], i16)
            nc.sync.dma_start(out=idx_sb[:], in_=idx16[:])
            xt_sb = cpool.tile([P, BC], f16)
            nc.sync.dma_start(out=xt_sb[:], in_=xt[:])
            vd_sb = cpool.tile([P, ROWP], f16)
            nc.sync.dma_start(out=vd_sb[:], in_=vd[:])
            id_sb = cpool.tile([P, P], f16)
            nc.sync.dma_start(out=id_sb[:], in_=ident[:])

            # field accumulators, one PSUM tile per 128-sample chunk
            psums = [ppool.tile([P, ROWP], f32, space="PSUM") for _ in range(NCHUNK)]

            for j in range(N_SPARSE):
                g = gpool.tile([P, NCHUNK, ROWP], f16, tag="g")
                base = N_DENSE + j * VOCAB
                nc.gpsimd.dma_gather(
                    out_ap=g[:],
                    in_ap=table[base:base + VOCAB, :],
                    idxs_ap=idx_sb[:, j * IDXC:(j + 1) * IDXC],
                    num_idxs=BC,
                    num_idxs_reg=BC,
                    elem_size=ROWP,
                    single_packet=True,
                    queue_num=j % nq,
                )
                for c in range(NCHUNK):
                    nc.tensor.matmul(
                        out=psums[c][:],
                        lhsT=id_sb[:],
                        rhs=g[:, c, :],
                        start=(j == 0),
                        stop=False,
                    )

            # dense contribution + linear terms into the same accumulators
            for c in range(NCHUNK):
                nc.tensor.matmul(
                    out=psums[c][:],
                    lhsT=xt_sb[:, c * P:(c + 1) * P],
                    rhs=vd_sb[:],
                    start=False,
                    stop=True,
                )

            for c in range(NCHUNK):
                field = psums[c]

                # q_half = 0.5 * sum(field[:, :312]^2)
                sq = wpool.tile([P, ROW], f32)
                qh = wpool.tile([P, 1], f32)
                nc.vector.tensor_tensor_reduce(
                    out=sq[:],
                    in0=field[:, :ROW],
                    in1=field[:, :ROW],
                    scale=0.5,
                    scalar=0.0,
                    op0=mybir.AluOpType.mult,
                    op1=mybir.AluOpType.add,
                    accum_out=qh[:],
                )

                # s[k] = sum_f field[f*8+k]: view [P, K, N_FIELD], reduce X
                s = wpool.tile([P, K], f32)
                fv = field[:, :ROW].rearrange("p (f k) -> p k f", f=N_FIELD)
                nc.vector.reduce_sum(out=s[:], in_=fv, axis=mybir.AxisListType.X)

                # ssum_half = 0.5 * sum_k s^2
                ss = wpool.tile([P, K], f32)
                sh = wpool.tile([P, 1], f32)
                nc.vector.tensor_tensor_reduce(
                    out=ss[:],
                    in0=s[:],
                    in1=s[:],
                    scale=0.5,
                    scalar=0.0,
                    op0=mybir.AluOpType.mult,
                    op1=mybir.AluOpType.add,
                    accum_out=sh[:],
                )

                # logit = cross + linear;  cross = ssum_half - q_half
                d = wpool.tile([P, 1], f32)
                nc.vector.tensor_tensor(
                    out=d[:], in0=sh[:], in1=qh[:], op=mybir.AluOpType.subtract
                )
                d2 = wpool.tile([P, 1], f32)
                nc.vector.tensor_tensor(
                    out=d2[:],
                    in0=d[:],
                    in1=field[:, ROW:ROW + 1],
                    op=mybir.AluOpType.add,
                )

                oc = wpool.tile([P, 1], f32)
                nc.scalar.activation(
                    oc[:], d2[:], mybir.ActivationFunctionType.Sigmoid
                )
                nc.sync.dma_start(out=out[c * P:(c + 1) * P, :], in_=oc[:])

    nc.compile()
    return nc


def _prep_inputs(dense_x, sparse_x, w0, w, v):
    table = np.zeros((N_FEAT, ROWP), dtype=np.float16)
    table[:, :ROW] = v.reshape(N_FEAT, ROW).astype(np.float16)
    table[:, ROW] = w[:, 0].astype(np.float16)

    vd = np.zeros((P, ROWP), dtype=np.float16)
    vd[:N_DENSE, :ROW] = v[:N_DENSE].reshape(N_DENSE, ROW).astype(np.float16)
    vd[:N_DENSE, ROW] = w[:N_DENSE, 0].astype(np.float16)
    vd[N_DENSE, ROW] = np.float16(w0[0])

    xt_full = np.zeros((P, BATCH), dtype=np.float16)
    xt_full[:N_DENSE] = dense_x.T.astype(np.float16)
    xt_full[N_DENSE] = 1.0

    ident = np.eye(P, dtype=np.float16)

    in_maps = []
    for r in range(N_CORES):
        b0 = r * BC
        sp = sparse_x[b0:b0 + BC].astype(np.int16)  # values < 10000 fit
        idx16 = np.zeros((P, N_SPARSE * IDXC), dtype=np.int16)
        for j in range(N_SPARSE):
            # gather position g reads idx[g%16, g//16]; g == sample index.
            # Replicated across all 8 GPSIMD-core partition groups (HW reads
            # its own 16-partition window).
            blk = sp[:, j].reshape(IDXC, 16).T
            idx16[:, j * IDXC:(j + 1) * IDXC] = np.tile(blk, (P // 16, 1))
        in_maps.append(
            {
                "table": table,
                "xt": np.ascontiguousarray(xt_full[:, b0:b0 + BC]),
                "vd": vd,
                "idx16": idx16,
                "ident": ident,
            }
        )
    return in_maps


def kernel(dense_x, sparse_x, w0, w, v, _trace=False, _trace_kwargs=None):
    from concourse.bass_utils import run_bass_kernel_spmd

    if "nc" not in _CACHE:
        _CACHE["nc"] = _build_program()
    nc = _CACHE["nc"]

    in_maps = _prep_inputs(dense_x, sparse_x, w0, w, v)
    kw = {}
    if _trace:
        kw["trace"] = True
        if _trace_kwargs:
            kw.update(_trace_kwargs)
    res = run_bass_kernel_spmd(nc, in_maps, core_ids=list(range(N_CORES)), **kw)
    outs = [res.results[r]["out"] for r in range(N_CORES)]
    full = np.concatenate(outs, axis=0).astype(np.float32)
    if _trace:
        _CACHE["last_exec_time_ns"] = res.exec_time_ns
        _CACHE["last_results"] = res
    return full
